# revision 1
# baseline (speedup 1.0000x reference)
"""Trainium2 Bass kernel for ConvNet forward (conv7x7s3 -> sq -> fc -> sq -> fc).

Strategy: pure data parallel over 8 NeuronCores (2048 samples each).
The conv is lowered to a block-sparse dense matrix A [324, 3072] applied via
float32r matmuls with batch as the moving free dim (N=512). Host pre-transposes
x to feature-major layout so no on-chip transpose is needed; fc1 weights are
permuted to match the conv-output row grouping; the [10, 2048] per-core output
is transposed back to [B, 10] on the host.
"""

import numpy as np

for _p in ("/opt/trn_rl_repo", "/root/.axon_site/_ro/trn_rl_repo"):
    try:
        import concourse  # noqa: F401
        break
    except ImportError:
        import sys
        if _p not in sys.path:
            sys.path.insert(0, _p)

# network constants
KERNEL, STRIDE = 7, 3
C_IN, C_OUT = 3, 4
HIDDEN, OUTPUT = 64, 10
H_OUT = 9                      # (32-7)//3 + 1
B_TOT, N_CORES = 16384, 8
B_CORE = B_TOT // N_CORES      # 2048
N_TILE = 512                   # batch tile (matmul moving free dim)
T_TILES = B_CORE // N_TILE     # 4
M_GROUP = 3 * C_OUT * H_OUT    # 108 outputs per i-group (3 rows x 4 ch x 9 cols)

# Only 31x31 of each 32x32 input image is read by the conv (stride 3, k=7):
# pack the 3*31*31 = 2883 used features densely, padded to 23 chunks of 128.
USED_IDX = np.array([ci * 1024 + r * 32 + w
                     for ci in range(C_IN) for r in range(31) for w in range(31)],
                    np.int64)
N_USED = len(USED_IDX)         # 2883
K_CHUNKS = 23
F_PACK = K_CHUNKS * 128        # 2944

# i-group g in {0,1,2} covers output rows i in {3g,3g+1,3g+2} and needs input
# rows r in [9g, 9g+12] of every channel. Chunk k feeds group g iff any packed
# feature in it lies in that row range.
GROUP_PAIRS = [[] for _ in range(3)]
for _k in range(K_CHUNKS):
    _rs = set()
    for _p in range(128 * _k, min(128 * (_k + 1), N_USED)):
        _rs.add((_p % 961) // 31)
    for _g in range(3):
        if any(9 * _g <= _r <= 9 * _g + 12 for _r in _rs):
            GROUP_PAIRS[_g].append(_k)
PAIRS = [(g, k) for g in range(3) for k in GROUP_PAIRS[g]]
N_PAIRS = len(PAIRS)


def _build_nc(repeat=1, mode="full"):
    import concourse.bacc as bacc
    import concourse.mybir as mybir
    from concourse.tile import TileContext

    F32R = mybir.dt.float32r
    F32 = mybir.dt.float32
    AF = mybir.ActivationFunctionType

    nc = bacc.Bacc()
    # partition-major pack: [t, p, c, n] so each partition's DMA read is a
    # single 48 KB contiguous run
    xT = nc.declare_dram_parameter(
        "xT", [T_TILES, 128, K_CHUNKS, N_TILE], F32R, isOutput=False)
    ATp = nc.declare_dram_parameter("ATp", [128, N_PAIRS * M_GROUP], F32R, isOutput=False)
    FC1 = nc.declare_dram_parameter("FC1", [M_GROUP, 3 * HIDDEN], F32R, isOutput=False)
    FC2 = nc.declare_dram_parameter("FC2", [HIDDEN, OUTPUT], F32R, isOutput=False)
    B1 = nc.declare_dram_parameter("B1", [HIDDEN, 1], F32, isOutput=False)
    B2 = nc.declare_dram_parameter("B2", [OUTPUT, 1], F32, isOutput=False)
    OUT = nc.declare_dram_parameter("OUT", [OUTPUT, B_CORE], F32, isOutput=True)

    with TileContext(nc) as tc:
        with tc.tile_pool(name="wpool", bufs=1) as wpool, \
             tc.tile_pool(name="xpool", bufs=3) as xpool, \
             tc.tile_pool(name="ypool", bufs=4) as ypool, \
             tc.tile_pool(name="opool", bufs=1) as opool, \
             tc.tile_pool(name="psy", bufs=4, space="PSUM") as psy, \
             tc.tile_pool(name="psh", bufs=2, space="PSUM") as psh, \
             tc.tile_pool(name="pso", bufs=2, space="PSUM") as pso:

            ats = wpool.tile([128, N_PAIRS * M_GROUP], F32R, tag="ats")
            fc1t = wpool.tile([M_GROUP, 3 * HIDDEN], F32R, tag="fc1t")
            fc2t = wpool.tile([HIDDEN, OUTPUT], F32R, tag="fc2t")
            b1t = wpool.tile([HIDDEN, 1], F32, tag="b1t")
            b2t = wpool.tile([OUTPUT, 1], F32, tag="b2t")
            nc.sync.dma_start(out=ats, in_=ATp[:, :])
            nc.sync.dma_start(out=fc1t, in_=FC1[:, :])
            nc.sync.dma_start(out=fc2t, in_=FC2[:, :])
            nc.sync.dma_start(out=b1t, in_=B1[:, :])
            nc.sync.dma_start(out=b2t, in_=B2[:, :])
            outsb = opool.tile([OUTPUT, B_CORE], F32, tag="outsb")
            if mode == "dma":
                nc.gpsimd.memset(outsb, 0.0)

            if mode == "compute":
                xt_fixed = xpool.tile([128, K_CHUNKS, N_TILE], F32R, tag="xt")
                nc.sync.dma_start(out=xt_fixed, in_=xT[0])
            for _rep in range(repeat):
                for t in range(T_TILES):
                    if mode == "compute":
                        xt = xt_fixed
                    else:
                        xt = xpool.tile([128, K_CHUNKS, N_TILE], F32R, tag="xt")
                        nc.sync.dma_start(out=xt, in_=xT[t])
                    if mode == "dma":
                        continue
                    y2 = []
                    pcnt = 0
                    for g in range(3):
                        ps = psy.tile([M_GROUP, N_TILE], F32, tag="psy")
                        ks = GROUP_PAIRS[g]
                        for idx, k in enumerate(ks):
                            nc.tensor.matmul(
                                ps,
                                ats[:, pcnt * M_GROUP:(pcnt + 1) * M_GROUP],
                                xt[:, k, :],
                                start=(idx == 0),
                                stop=(idx == len(ks) - 1),
                            )
                            pcnt += 1
                        yt = ypool.tile([M_GROUP, N_TILE], F32R, tag="y2")
                        nc.scalar.activation(yt, ps, AF.Square)
                        y2.append(yt)
                    hp = psh.tile([HIDDEN, N_TILE], F32, tag="psh")
                    for g in range(3):
                        nc.tensor.matmul(
                            hp,
                            fc1t[:, g * HIDDEN:(g + 1) * HIDDEN],
                            y2[g],
                            start=(g == 0),
                            stop=(g == 2),
                        )
                    h2 = ypool.tile([HIDDEN, N_TILE], F32R, tag="h2")
                    nc.scalar.activation(h2, hp, AF.Square, bias=b1t)
                    op = pso.tile([OUTPUT, N_TILE], F32, tag="pso")
                    nc.tensor.matmul(op, fc2t, h2, start=True, stop=True)
                    nc.scalar.activation(
                        outsb[:, t * N_TILE:(t + 1) * N_TILE], op, AF.Identity,
                        bias=b2t,
                    )
            nc.sync.dma_start(out=OUT[:, :], in_=outsb)
    nc.finalize()
    return nc


def _prep_weights(conv_w, fc1_w, fc1_b, fc2_w, fc2_b):
    # A[g, local, f]: dense conv matrix split by i-group.
    # local = il*36 + c*9 + j  (i = 3g+il), f = ci*1024 + r*32 + w
    A = np.zeros((3, M_GROUP, C_IN * 1024), np.float32)
    for g in range(3):
        for il in range(3):
            i = 3 * g + il
            for c in range(C_OUT):
                for j in range(H_OUT):
                    row = il * 36 + c * 9 + j
                    for ci in range(C_IN):
                        for ki in range(KERNEL):
                            f0 = ci * 1024 + (3 * i + ki) * 32 + 3 * j
                            A[g, row, f0:f0 + KERNEL] = conv_w[c, ci, ki, :]
    # gather used feature columns, zero-pad to F_PACK
    Ap = np.zeros((3, M_GROUP, F_PACK), np.float32)
    Ap[:, :, :N_USED] = A[:, :, USED_IDX]
    # pack the active [128, 108] transposed blocks side by side
    ATp = np.empty((128, N_PAIRS * M_GROUP), np.float32)
    for p, (g, k) in enumerate(PAIRS):
        ATp[:, p * M_GROUP:(p + 1) * M_GROUP] = Ap[g, :, 128 * k:128 * (k + 1)].T
    # fc1 columns permuted to our y-row order: global y row g*108+il*36+c*9+j
    # corresponds to reference flat index c*81 + (3g+il)*9 + j
    gg, ll, cc, jj = np.meshgrid(np.arange(3), np.arange(3), np.arange(C_OUT),
                                 np.arange(H_OUT), indexing="ij")
    orig = (cc * 81 + (3 * gg + ll) * 9 + jj).reshape(-1)
    fc1p = fc1_w[:, orig].T.astype(np.float32)        # [324, 64]
    FC1 = np.empty((M_GROUP, 3 * HIDDEN), np.float32)
    for g in range(3):
        FC1[:, g * HIDDEN:(g + 1) * HIDDEN] = fc1p[g * M_GROUP:(g + 1) * M_GROUP]
    FC2 = np.ascontiguousarray(fc2_w.T.astype(np.float32))  # [64, 10]
    B1 = np.ascontiguousarray(fc1_b.reshape(HIDDEN, 1).astype(np.float32))
    B2 = np.ascontiguousarray(fc2_b.reshape(OUTPUT, 1).astype(np.float32))
    return ATp, FC1, FC2, B1, B2


def _make_in_maps(x, ATp, FC1, FC2, B1, B2):
    in_maps = []
    xf = x.reshape(B_TOT, C_IN * 1024)
    for c in range(N_CORES):
        xs = xf[c * B_CORE:(c + 1) * B_CORE]
        xg = np.zeros((B_CORE, F_PACK), np.float32)
        xg[:, :N_USED] = xs[:, USED_IDX]
        xg = xg.reshape(T_TILES, N_TILE, K_CHUNKS, 128)
        xTc = np.ascontiguousarray(xg.transpose(0, 3, 2, 1))  # [4, 128, 23, 512]
        in_maps.append({"xT": xTc, "ATp": ATp, "FC1": FC1, "FC2": FC2,
                        "B1": B1, "B2": B2})
    return in_maps


def kernel(x, conv_w, fc1_w, fc1_b, fc2_w, fc2_b):
    from concourse.bass_utils import run_bass_kernel_spmd

    x = np.asarray(x, np.float32)
    ATp, FC1, FC2, B1, B2 = _prep_weights(
        np.asarray(conv_w, np.float32), np.asarray(fc1_w, np.float32),
        np.asarray(fc1_b, np.float32), np.asarray(fc2_w, np.float32),
        np.asarray(fc2_b, np.float32))

    in_maps = _make_in_maps(x, ATp, FC1, FC2, B1, B2)

    nc = _build_nc(repeat=1)
    res = run_bass_kernel_spmd(nc, in_maps, list(range(N_CORES)))
    out = np.empty((B_TOT, OUTPUT), np.float32)
    for c in range(N_CORES):
        out[c * B_CORE:(c + 1) * B_CORE] = res.results[c]["OUT"].T
    return out



# revision 2
# speedup vs baseline: 2.4777x; 2.4777x over previous
"""Trainium2 Bass kernel for ConvNet forward (conv7x7s3 -> sq -> fc -> sq -> fc).

Strategy: pure data parallel over 8 NeuronCores (2048 samples each).
The conv is lowered to a block-sparse dense matrix applied via matmuls with
batch as the moving free dim (N=512). x is shipped as float16 (the 2e-2
tolerance leaves ample room), packed on the host in row-block-major feature
order so each of the 3 output-row groups reads a CONTIGUOUS span of packed
features: group supports become 10 aligned 128-chunks each (30 chunk-matmuls
per batch tile vs 36 for naive row-major packing). Only the 2883 used
features are transferred (22 full 128-chunks + one 67-partition tail chunk —
the zero pad is never DMA'd). The fc pipeline stays float32r. The [10, 2048]
per-core output is transposed back to [B, 10] on the host.
"""

import numpy as np

for _p in ("/opt/trn_rl_repo", "/root/.axon_site/_ro/trn_rl_repo"):
    try:
        import concourse  # noqa: F401
        break
    except ImportError:
        import sys
        if _p not in sys.path:
            sys.path.insert(0, _p)

# network constants
KERNEL, STRIDE = 7, 3
C_IN, C_OUT = 3, 4
HIDDEN, OUTPUT = 64, 10
H_OUT = 9                      # (32-7)//3 + 1
B_TOT, N_CORES = 16384, 8
B_CORE = B_TOT // N_CORES      # 2048
N_TILE = 512                   # batch tile (matmul moving free dim)
T_TILES = B_CORE // N_TILE     # 4
M_GROUP = 3 * C_OUT * H_OUT    # 108 outputs per i-group (3 rows x 4 ch x 9 cols)

# Only 31x31 of each 32x32 input image is read by the conv (stride 3, k=7).
# i-group g in {0,1,2} covers output rows i in {3g..3g+2} and needs input rows
# [9g, 9g+12]. Pack features row-block-major — blocks R0=rows 0-8, R1=9-12,
# R2=13-17, R3=18-21, R4=22-30, channels-major inside a block — so each
# group's support is one contiguous span:
#   g0 = R0+R1 = [0, 1209)   g1 = R1+R2+R3 = [837, 2046)   g2 = R3+R4 = [1674, 2883)
ROW_BLOCKS = [(0, 9), (9, 13), (13, 18), (18, 22), (22, 31)]
USED_IDX = np.array([ci * 1024 + r * 32 + w
                     for (r0, r1) in ROW_BLOCKS
                     for ci in range(C_IN) for r in range(r0, r1)
                     for w in range(31)], np.int64)
N_USED = len(USED_IDX)         # 2883
K_CHUNKS = 23                  # ceil(2883/128)
K_MAIN = 22                    # full 128-partition chunks
TAIL = N_USED - 128 * K_MAIN   # 67 partitions in the last chunk
F_PACK = K_CHUNKS * 128        # 2944

_BS = 31 * C_IN                # features per row (all channels): 93
_SUPPORT = [(0, 13 * _BS), (9 * _BS, 22 * _BS), (18 * _BS, 31 * _BS)]
GROUP_PAIRS = [[k for k in range(K_CHUNKS)
                if 128 * k < e and 128 * (k + 1) > s]
               for (s, e) in _SUPPORT]
PAIRS = [(g, k) for g in range(3) for k in GROUP_PAIRS[g]]
N_PAIRS = len(PAIRS)           # 30


def _build_nc(repeat=1, mode="full"):
    import concourse.bacc as bacc
    import concourse.mybir as mybir
    from concourse.tile import TileContext

    F32R = mybir.dt.float32r
    F32 = mybir.dt.float32
    F16 = mybir.dt.float16
    AF = mybir.ActivationFunctionType

    nc = bacc.Bacc()
    # partition-major pack: per-partition DMA reads are single contiguous runs
    xA = nc.declare_dram_parameter(
        "xA", [T_TILES, 128, K_MAIN, N_TILE], F16, isOutput=False)
    xB = nc.declare_dram_parameter(
        "xB", [T_TILES, TAIL, N_TILE], F16, isOutput=False)
    ATp = nc.declare_dram_parameter("ATp", [128, N_PAIRS * M_GROUP], F16, isOutput=False)
    FC1 = nc.declare_dram_parameter("FC1", [M_GROUP, 3 * HIDDEN], F32R, isOutput=False)
    FC2 = nc.declare_dram_parameter("FC2", [HIDDEN, OUTPUT], F32R, isOutput=False)
    B1 = nc.declare_dram_parameter("B1", [HIDDEN, 1], F32, isOutput=False)
    B2 = nc.declare_dram_parameter("B2", [OUTPUT, 1], F32, isOutput=False)
    OUT = nc.declare_dram_parameter("OUT", [OUTPUT, B_CORE], F32, isOutput=True)

    with TileContext(nc) as tc:
        with tc.tile_pool(name="wpool", bufs=1) as wpool, \
             tc.tile_pool(name="xpool", bufs=3) as xpool, \
             tc.tile_pool(name="ypool", bufs=4) as ypool, \
             tc.tile_pool(name="opool", bufs=1) as opool, \
             tc.tile_pool(name="psy", bufs=4, space="PSUM") as psy, \
             tc.tile_pool(name="psh", bufs=2, space="PSUM") as psh, \
             tc.tile_pool(name="pso", bufs=2, space="PSUM") as pso:

            ats = wpool.tile([128, N_PAIRS * M_GROUP], F16, tag="ats")
            fc1t = wpool.tile([M_GROUP, 3 * HIDDEN], F32R, tag="fc1t")
            fc2t = wpool.tile([HIDDEN, OUTPUT], F32R, tag="fc2t")
            b1t = wpool.tile([HIDDEN, 1], F32, tag="b1t")
            b2t = wpool.tile([OUTPUT, 1], F32, tag="b2t")
            nc.sync.dma_start(out=ats, in_=ATp[:, :])
            nc.sync.dma_start(out=fc1t, in_=FC1[:, :])
            nc.sync.dma_start(out=fc2t, in_=FC2[:, :])
            nc.sync.dma_start(out=b1t, in_=B1[:, :])
            nc.sync.dma_start(out=b2t, in_=B2[:, :])
            outsb = opool.tile([OUTPUT, B_CORE], F32, tag="outsb")
            if mode == "dma":
                nc.gpsimd.memset(outsb, 0.0)

            if mode == "compute":
                xt_fixed = xpool.tile([128, K_MAIN, N_TILE], F16, tag="xt")
                xb_fixed = xpool.tile([TAIL, N_TILE], F16, tag="xb")
                nc.sync.dma_start(out=xt_fixed, in_=xA[0])
                nc.sync.dma_start(out=xb_fixed, in_=xB[0])
            for _rep in range(repeat):
                for t in range(T_TILES):
                    if mode == "compute":
                        xt, xb = xt_fixed, xb_fixed
                    else:
                        xt = xpool.tile([128, K_MAIN, N_TILE], F16, tag="xt")
                        xb = xpool.tile([TAIL, N_TILE], F16, tag="xb")
                        nc.sync.dma_start(out=xt, in_=xA[t])
                        nc.sync.dma_start(out=xb, in_=xB[t])
                    if mode == "dma":
                        continue
                    y2 = []
                    pcnt = 0
                    for g in range(3):
                        ps = psy.tile([M_GROUP, N_TILE], F32, tag="psy")
                        ks = GROUP_PAIRS[g]
                        for idx, k in enumerate(ks):
                            cols = slice(pcnt * M_GROUP, (pcnt + 1) * M_GROUP)
                            if k == K_MAIN:
                                nc.tensor.matmul(
                                    ps, ats[0:TAIL, cols], xb[0:TAIL, :],
                                    start=(idx == 0), stop=(idx == len(ks) - 1))
                            else:
                                nc.tensor.matmul(
                                    ps, ats[:, cols], xt[:, k, :],
                                    start=(idx == 0), stop=(idx == len(ks) - 1))
                            pcnt += 1
                        yt = ypool.tile([M_GROUP, N_TILE], F32R, tag="y2")
                        nc.scalar.activation(yt, ps, AF.Square)
                        y2.append(yt)
                    hp = psh.tile([HIDDEN, N_TILE], F32, tag="psh")
                    for g in range(3):
                        nc.tensor.matmul(
                            hp,
                            fc1t[:, g * HIDDEN:(g + 1) * HIDDEN],
                            y2[g],
                            start=(g == 0),
                            stop=(g == 2),
                        )
                    h2 = ypool.tile([HIDDEN, N_TILE], F32R, tag="h2")
                    nc.scalar.activation(h2, hp, AF.Square, bias=b1t)
                    op = pso.tile([OUTPUT, N_TILE], F32, tag="pso")
                    nc.tensor.matmul(op, fc2t, h2, start=True, stop=True)
                    nc.scalar.activation(
                        outsb[:, t * N_TILE:(t + 1) * N_TILE], op, AF.Identity,
                        bias=b2t,
                    )
            nc.sync.dma_start(out=OUT[:, :], in_=outsb)
    nc.finalize()
    return nc


def _prep_weights(conv_w, fc1_w, fc1_b, fc2_w, fc2_b):
    # A[g, local, f]: dense conv matrix split by i-group.
    # local = il*36 + c*9 + j  (i = 3g+il), f = ci*1024 + r*32 + w
    A = np.zeros((3, M_GROUP, C_IN * 1024), np.float32)
    for g in range(3):
        for il in range(3):
            i = 3 * g + il
            for c in range(C_OUT):
                for j in range(H_OUT):
                    row = il * 36 + c * 9 + j
                    for ci in range(C_IN):
                        for ki in range(KERNEL):
                            f0 = ci * 1024 + (3 * i + ki) * 32 + 3 * j
                            A[g, row, f0:f0 + KERNEL] = conv_w[c, ci, ki, :]
    # gather used feature columns (row-block-major order), zero-pad to F_PACK
    Ap = np.zeros((3, M_GROUP, F_PACK), np.float32)
    Ap[:, :, :N_USED] = A[:, :, USED_IDX]
    # pack the active [128, 108] transposed blocks side by side
    ATp = np.empty((128, N_PAIRS * M_GROUP), np.float16)
    for p, (g, k) in enumerate(PAIRS):
        ATp[:, p * M_GROUP:(p + 1) * M_GROUP] = Ap[g, :, 128 * k:128 * (k + 1)].T
    # fc1 columns permuted to our y-row order: global y row g*108+il*36+c*9+j
    # corresponds to reference flat index c*81 + (3g+il)*9 + j
    gg, ll, cc, jj = np.meshgrid(np.arange(3), np.arange(3), np.arange(C_OUT),
                                 np.arange(H_OUT), indexing="ij")
    orig = (cc * 81 + (3 * gg + ll) * 9 + jj).reshape(-1)
    fc1p = fc1_w[:, orig].T.astype(np.float32)        # [324, 64]
    FC1 = np.empty((M_GROUP, 3 * HIDDEN), np.float32)
    for g in range(3):
        FC1[:, g * HIDDEN:(g + 1) * HIDDEN] = fc1p[g * M_GROUP:(g + 1) * M_GROUP]
    FC2 = np.ascontiguousarray(fc2_w.T.astype(np.float32))  # [64, 10]
    B1 = np.ascontiguousarray(fc1_b.reshape(HIDDEN, 1).astype(np.float32))
    B2 = np.ascontiguousarray(fc2_b.reshape(OUTPUT, 1).astype(np.float32))
    return ATp, FC1, FC2, B1, B2


def _make_in_maps(x, ATp, FC1, FC2, B1, B2):
    in_maps = []
    xf = x.reshape(B_TOT, C_IN * 1024)
    for c in range(N_CORES):
        xs = xf[c * B_CORE:(c + 1) * B_CORE]
        xg = xs[:, USED_IDX].astype(np.float16)       # [2048, 2883]
        xg = xg.reshape(T_TILES, N_TILE, N_USED)
        main = xg[:, :, :128 * K_MAIN].reshape(T_TILES, N_TILE, K_MAIN, 128)
        xAc = np.ascontiguousarray(main.transpose(0, 3, 2, 1))  # [4, 128, 22, 512]
        xBc = np.ascontiguousarray(
            xg[:, :, 128 * K_MAIN:].transpose(0, 2, 1))         # [4, 67, 512]
        in_maps.append({"xA": xAc, "xB": xBc, "ATp": ATp, "FC1": FC1,
                        "FC2": FC2, "B1": B1, "B2": B2})
    return in_maps


def kernel(x, conv_w, fc1_w, fc1_b, fc2_w, fc2_b):
    from concourse.bass_utils import run_bass_kernel_spmd

    x = np.asarray(x, np.float32)
    ATp, FC1, FC2, B1, B2 = _prep_weights(
        np.asarray(conv_w, np.float32), np.asarray(fc1_w, np.float32),
        np.asarray(fc1_b, np.float32), np.asarray(fc2_w, np.float32),
        np.asarray(fc2_b, np.float32))

    in_maps = _make_in_maps(x, ATp, FC1, FC2, B1, B2)

    nc = _build_nc(repeat=1)
    res = run_bass_kernel_spmd(nc, in_maps, list(range(N_CORES)))
    out = np.empty((B_TOT, OUTPUT), np.float32)
    for c in range(N_CORES):
        out[c * B_CORE:(c + 1) * B_CORE] = res.results[c]["OUT"].T
    return out
